# revision 5
# baseline (speedup 1.0000x reference)
"""Trainium2 Bass kernel for causal self-attention with RoPE (Megatron-style
head-parallel over 8 NeuronCores).

Sharding: 16 heads / 8 cores = 2 heads per core. Wqkv split column-wise by
head; attention embarrassingly parallel over (batch, head); output projection
row-parallel with the partial contraction exchanged via per-batch AllToAlls
(4 collectives, each launched as soon as that batch's attention completes, so
the exchange overlaps attention/projection compute). Core r ends up owning a
128-token strip of each batch: tokens [b*1024 + r*128, b*1024 + (r+1)*128).

All matmuls bf16 with fp32 PSUM accumulation. Softmax skips max-subtraction
(scores are O(+-10) here). The denominator is computed off the PE: exp chunks
are accumulated on the DVE and partition-reduced on GpSimd. Score matmuls and
exp are trimmed to the causally-live column range (the masked tail is zeroed
by the existing diagonal-block masks). RoPE rotate-half runs as a +-1
permutation matmul on the PE.
"""

import sys

if "/opt/trn_rl_repo" not in sys.path:
    sys.path.insert(0, "/opt/trn_rl_repo")

import ml_dtypes
import numpy as np

import concourse.bacc as bacc
import concourse.bass as bass
import concourse.bass_isa as bass_isa
import concourse.mybir as mybir
import concourse.tile as tile
from concourse.bass_utils import run_bass_kernel_spmd

B, T, C, H, D = 4, 1024, 2048, 16, 128
TQ = B * T
NCORES = 8
HPC = H // NCORES    # heads per core = 2
FQK = 4 * D          # 512 qkT feature rows per core (qa, qb, ka, kb)
FV = HPC * D         # 256 v feature cols per core
STRIP = T // NCORES  # 128 tokens per (core, batch)
NCT = C // 128       # 16 contraction tiles
SCALE = 1.0 / float(np.sqrt(D))

F32 = mybir.dt.float32
BF16 = mybir.dt.bfloat16

_CACHE = {}


def _build_program():
    nc = bacc.Bacc(
        "TRN2",
        target_bir_lowering=False,
        debug=False,
        enable_asserts=False,
        num_devices=NCORES,
    )

    # ---- I/O -----------------------------------------------------------
    xT = nc.dram_tensor("xT", [C, TQ], BF16, kind="ExternalInput")
    wqk = nc.dram_tensor("wqk", [C, FQK], BF16, kind="ExternalInput")
    wv = nc.dram_tensor("wv", [C, FV], BF16, kind="ExternalInput")
    bqk = nc.dram_tensor("bqk", [128, 4], F32, kind="ExternalInput")
    bv = nc.dram_tensor("bv", [128, FV], F32, kind="ExternalInput")
    wproj = nc.dram_tensor("wproj", [C, C], BF16, kind="ExternalInput")
    bproj = nc.dram_tensor("bproj", [128, C], F32, kind="ExternalInput")
    cosd = nc.dram_tensor("cosd", [128, T], BF16, kind="ExternalInput")
    sind = nc.dram_tensor("sind", [128, T], BF16, kind="ExternalInput")
    rmat = nc.dram_tensor("rmat", [128, 128], BF16, kind="ExternalInput")
    out = nc.dram_tensor("out", [B * STRIP, C], F32, kind="ExternalOutput")

    Exp = mybir.ActivationFunctionType.Exp
    add = mybir.AluOpType.add
    mult = mybir.AluOpType.mult
    radd = bass_isa.ReduceOp.add

    with tile.TileContext(nc) as tc:
        with (
            tc.tile_pool(name="const", bufs=1) as cpool,
            tc.tile_pool(name="act", bufs=2) as act,
            tc.tile_pool(name="work", bufs=2) as wpool,
            tc.tile_pool(name="att", bufs=2) as apool,
            tc.tile_pool(name="psA", bufs=2, space="PSUM") as psA,
            tc.tile_pool(name="psB", bufs=2, space="PSUM") as psB,
            tc.tile_pool(name="dram", bufs=1, space="DRAM") as dpool,
        ):
            # ---- initial loads: wqk on gpsimd, x chunk 0 on sync, ------
            # constants + wproj on scalar. Fine-grained first pieces so
            # the first accumulation group starts ASAP.
            wqk_sb = cpool.tile([128, NCT, FQK], BF16)
            wqk_r = wqk.rearrange("(ct p) f -> p ct f", p=128)
            xt_tiles = {}
            xt_tiles[0] = wpool.tile(
                [128, NCT, 512], BF16, tag="xT_ch", name="xT_ch0"
            )
            xT_r0 = xT[:, 0:512].rearrange("(ct p) t -> p ct t", p=128)
            for pc in range(4):
                s = slice(pc * 4, (pc + 1) * 4)
                nc.gpsimd.dma_start(out=wqk_sb[:, s, :], in_=wqk_r[:, s, :])
                nc.sync.dma_start(out=xt_tiles[0][:, s, :], in_=xT_r0[:, s, :])
            wv_sb = cpool.tile([128, NCT, FV], BF16)
            wv_r = wv.rearrange("(ct p) f -> p ct f", p=128)
            for pc in range(2):
                s = slice(pc * 8, (pc + 1) * 8)
                nc.gpsimd.dma_start(out=wv_sb[:, s, :], in_=wv_r[:, s, :])

            # small constants on the scalar queue
            bqk_sb = cpool.tile([128, 4], F32)
            nc.scalar.dma_start(out=bqk_sb[:], in_=bqk[:])
            cos_sb = cpool.tile([128, T], BF16)
            nc.scalar.dma_start(out=cos_sb[:], in_=cosd[:])
            sin_sb = cpool.tile([128, T], BF16)
            nc.scalar.dma_start(out=sin_sb[:], in_=sind[:])
            rmat_sb = cpool.tile([128, 128], BF16)
            nc.scalar.dma_start(out=rmat_sb[:], in_=rmat[:])
            bv_sb = cpool.tile([128, FV], F32)
            nc.scalar.dma_start(out=bv_sb[:], in_=bv[:])
            bproj_sb = cpool.tile([128, C], F32)
            nc.scalar.dma_start(out=bproj_sb[:], in_=bproj[:])

            # Wproj chunks: all 4 resident; loaded on scalar queue during
            # phases 1-2, well before first use in proj(0).
            wp_tiles = {}
            for ec in range(4):
                wpt = cpool.tile([128, NCT, 512], BF16, name=f"wp{ec}")
                nc.scalar.dma_start(
                    out=wpt[:],
                    in_=wproj[:, ec * 512 : (ec + 1) * 512].rearrange(
                        "(ft p) e -> p ft e", p=128
                    ),
                )
                wp_tiles[ec] = wpt

            # diagonal-block masks: mask_m[p, col] = 1 if col >= p + 128*m
            mask_sb = cpool.tile([128, 4, 512], BF16)
            nc.gpsimd.memset(mask_sb[:], 1.0)
            for m in range(4):
                nc.gpsimd.affine_select(
                    out=mask_sb[:, m, :],
                    in_=mask_sb[:, m, :],
                    compare_op=mybir.AluOpType.is_ge,
                    fill=0.0,
                    base=-128 * m,
                    pattern=[[1, 512]],
                    channel_multiplier=-1,
                )
            # one-time zero of the 3 rotating exp-output buffers so stale
            # data in causally-trimmed (never-written) columns is finite;
            # the diagonal masks multiplicatively zero those columns.
            pt_init = []
            for i in range(3):
                t_ = apool.tile(
                    [128, 2, 512], BF16, tag="pt", bufs=3, name=f"ptz{i}"
                )
                nc.gpsimd.memset(t_[:], 0.0)
                pt_init.append(t_)

            # a2a buffers: per batch, slot p carries my 2 heads' yT for
            # token strip p of that batch (128 tokens).
            a2a_in = [
                dpool.tile([NCORES, FV, STRIP], BF16, name=f"a2a_in{b}")
                for b in range(B)
            ]
            a2a_out = [
                dpool.tile([NCORES, FV, STRIP], BF16, name=f"a2a_out{b}")
                for b in range(B)
            ]

            qk_tiles = {}
            v_tiles = {}

            # ---- phase 1 per batch: QKV projection + RoPE -------------
            def qkv(b):
                qkT_b = act.tile(
                    [128, 4, T], BF16, tag="qkT", name=f"qkT{b}"
                )
                v_b = act.tile(
                    [128, T // 128, FV], BF16, tag="vsb", name=f"v{b}"
                )
                qk_tiles[b] = qkT_b
                v_tiles[b] = v_b
                for cc in range(2):
                    ch = 2 * b + cc
                    t0 = cc * 512
                    if ch in xt_tiles:
                        xT_ch = xt_tiles[ch]
                    else:
                        xT_ch = wpool.tile(
                            [128, NCT, 512], BF16, tag="xT_ch",
                            name=f"xT_ch{ch}",
                        )
                        xt_tiles[ch] = xT_ch
                        g0 = ch * 512
                        xT_r = xT[:, g0 : g0 + 512].rearrange(
                            "(ct p) t -> p ct t", p=128
                        )
                        for pc in range(2):
                            s = slice(pc * 8, (pc + 1) * 8)
                            nc.sync.dma_start(
                                out=xT_ch[:, s, :], in_=xT_r[:, s, :]
                            )
                    for mi in range(4):
                        ps = psA.tile([128, 2, 512], F32, tag="mm512")
                        for ct in range(NCT):
                            nc.tensor.matmul(
                                ps[:, 0, :],
                                lhsT=wqk_sb[:, ct, mi * 128 : (mi + 1) * 128],
                                rhs=xT_ch[:, ct, :],
                                start=(ct == 0),
                                stop=(ct == NCT - 1),
                            )
                        # evict + bias + RoPE; rotate-half via +-1
                        # permutation matmul: dst = (ps+b)*cos + R^T@((ps+b)*sin)
                        m1 = wpool.tile([128, 512], BF16, tag="rope_m1")
                        m2 = wpool.tile([128, 512], BF16, tag="rope_m2")
                        nc.vector.scalar_tensor_tensor(
                            out=m2[:], in0=ps[:, 0, :],
                            scalar=bqk_sb[:, mi : mi + 1],
                            in1=sin_sb[:, t0 : t0 + 512], op0=add, op1=mult,
                        )
                        rot_ps = psB.tile([128, 512], F32, tag="aux")
                        nc.tensor.matmul(
                            rot_ps[:], lhsT=rmat_sb[:], rhs=m2[:],
                            start=True, stop=True,
                        )
                        nc.vector.scalar_tensor_tensor(
                            out=m1[:], in0=ps[:, 0, :],
                            scalar=bqk_sb[:, mi : mi + 1],
                            in1=cos_sb[:, t0 : t0 + 512], op0=add, op1=mult,
                        )
                        dst = qkT_b[:, mi, t0 : t0 + 512]
                        nc.vector.tensor_add(dst, m1[:], rot_ps[:])
                    for tt in range(4):
                        psv = psB.tile([128, FV], F32, tag="acc")
                        for ct in range(NCT):
                            nc.tensor.matmul(
                                psv[:],
                                lhsT=xT_ch[:, ct, tt * 128 : (tt + 1) * 128],
                                rhs=wv_sb[:, ct, :],
                                start=(ct == 0),
                                stop=(ct == NCT - 1),
                            )
                        nc.vector.tensor_add(
                            v_b[:, cc * 4 + tt, :], psv[:], bv_sb[:]
                        )

            # ---- phase 2 per batch: attention + per-batch AllToAll ----
            def att(b):
                qkT_b = qk_tiles[b]
                v_b = v_tiles[b]
                for hl in range(HPC):
                    qh = qkT_b[:, hl, :]
                    kh = qkT_b[:, 2 + hl, :]
                    for tqc in range(2):
                        q0 = tqc * 512
                        nj = 4 * (tqc + 1)
                        ot_ps = psB.tile([128, 512], F32, tag="acc")
                        sacc = apool.tile([128, 512], F32, tag="sacc")
                        first = True
                        for jp in range(nj // 2):
                            js = [2 * jp, 2 * jp + 1]
                            md = min(
                                max(0, 128 * (j - (nj - 4))) for j in js
                            )
                            st_ps = psA.tile([128, 2, 512], F32, tag="mm512")
                            for jj, j in enumerate(js):
                                s0 = j * 128
                                nc.tensor.matmul(
                                    st_ps[:, jj, md:512],
                                    lhsT=kh[:, s0 : s0 + 128],
                                    rhs=qh[:, q0 + md : q0 + 512],
                                    start=True,
                                    stop=True,
                                )
                            ptp = apool.tile(
                                [128, 2, 512], BF16, tag="pt", bufs=3
                            )
                            nc.scalar.activation(
                                ptp[:, :, md:512], st_ps[:, :, md:512],
                                Exp, scale=SCALE,
                            )
                            for jj, j in enumerate(js):
                                m = j - (nj - 4)
                                if m >= 0:
                                    nc.vector.tensor_mul(
                                        ptp[:, jj, :], ptp[:, jj, :],
                                        mask_sb[:, m, :],
                                    )
                            if first:
                                nc.vector.tensor_add(
                                    sacc[:], ptp[:, 0, :], ptp[:, 1, :]
                                )
                                first = False
                            else:
                                nc.vector.tensor_add(
                                    sacc[:], sacc[:], ptp[:, 0, :]
                                )
                                nc.vector.tensor_add(
                                    sacc[:], sacc[:], ptp[:, 1, :]
                                )
                            for jj, j in enumerate(js):
                                vt = v_b[:, j, hl * 128 : (hl + 1) * 128]
                                nc.tensor.matmul(
                                    ot_ps[:], lhsT=vt, rhs=ptp[:, jj, :],
                                    start=(j == 0), stop=(j == nj - 1),
                                )
                        den = apool.tile([128, 512], F32, tag="den")
                        nc.gpsimd.partition_all_reduce(
                            den[:], sacc[:], channels=128, reduce_op=radd
                        )
                        recipb = apool.tile([128, 512], F32, tag="recipb")
                        nc.vector.reciprocal_approx_fast(recipb[:], den[:])
                        yt = apool.tile([128, 512], BF16, tag="yt")
                        nc.vector.tensor_mul(yt[:], ot_ps[:], recipb[:])
                        # one DMA: strips of 128 tokens -> a2a slots
                        dst = a2a_in[b][
                            tqc * 4 : (tqc + 1) * 4,
                            hl * 128 : (hl + 1) * 128,
                            :,
                        ].rearrange("s d t -> d s t")
                        nc.scalar.dma_start(
                            out=dst,
                            in_=yt[:].rearrange("d (s t) -> d s t", s=4),
                        )
                nc.gpsimd.collective_compute(
                    "AllToAll",
                    mybir.AluOpType.bypass,
                    replica_groups=[list(range(NCORES))],
                    ins=[a2a_in[b][:].opt()],
                    outs=[a2a_out[b][:].opt()],
                )

            # ---- phase 3 per batch: projection of my 128-token strip --
            def proj(b):
                yts = wpool.tile(
                    [128, NCT, STRIP], BF16, tag="yts", name=f"yts{b}"
                )
                nc.sync.dma_start(
                    out=yts[:],
                    in_=a2a_out[b].rearrange("g (f2 p) t -> p (g f2) t", p=128),
                )
                for ec in range(4):
                    e0 = ec * 512
                    pps = psA.tile([128, 2, 512], F32, tag="mm512")
                    for ft in range(NCT):
                        nc.tensor.matmul(
                            pps[:, 0, :],
                            lhsT=yts[:, ft, :],
                            rhs=wp_tiles[ec][:, ft, :],
                            start=(ft == 0),
                            stop=(ft == NCT - 1),
                        )
                    osb = wpool.tile([128, 512], F32, tag="osb")
                    nc.vector.tensor_add(
                        osb[:], pps[:, 0, :], bproj_sb[:, e0 : e0 + 512]
                    )
                    nc.sync.dma_start(
                        out=out[b * STRIP : (b + 1) * STRIP, e0 : e0 + 512],
                        in_=osb[:],
                    )

            # ---- schedule: pipeline so each a2a overlaps later work ---
            qkv(0)
            qkv(1)
            att(0)
            qkv(2)
            att(1)
            proj(0)
            qkv(3)
            att(2)
            proj(1)
            att(3)
            proj(2)
            proj(3)

    nc.compile()
    return nc


def _rope_tables():
    inv = 1.0 / (10000.0 ** (np.arange(0, D, 2, dtype=np.float64) / D))
    t = np.arange(T, dtype=np.float64)
    fr = np.outer(t, inv)  # [T, 64]
    cosT = np.tile(np.cos(fr).T, (2, 1)).astype(ml_dtypes.bfloat16)
    sinT = np.tile(np.sin(fr).T, (2, 1)).astype(ml_dtypes.bfloat16)
    return np.ascontiguousarray(cosT), np.ascontiguousarray(sinT)


def _prep_inputs(x, Wqkv, bqkv, Wproj, bproj):
    x = np.asarray(x, np.float32).reshape(TQ, C)
    Wqkv = np.asarray(Wqkv, np.float32)
    bqkv = np.asarray(bqkv, np.float32)
    Wproj = np.ascontiguousarray(
        np.asarray(Wproj, np.float32).astype(ml_dtypes.bfloat16)
    )
    bproj = np.asarray(bproj, np.float32)

    xT = np.ascontiguousarray(x.T.astype(ml_dtypes.bfloat16))
    cosT, sinT = _rope_tables()
    rmat = np.zeros((128, 128), ml_dtypes.bfloat16)
    for i in range(64):
        rmat[64 + i, i] = -1.0   # out[p<64]  = -m2[p+64]
        rmat[i, 64 + i] = 1.0    # out[p>=64] = +m2[p-64]
    bproj_b = np.ascontiguousarray(np.broadcast_to(bproj[None, :], (128, C)))

    Wq = Wqkv[:, 0 * C : 1 * C].reshape(C, H, D)
    Wk = Wqkv[:, 1 * C : 2 * C].reshape(C, H, D)
    Wv = Wqkv[:, 2 * C : 3 * C].reshape(C, H, D)
    bq = bqkv[0 * C : 1 * C].reshape(H, D)
    bk = bqkv[1 * C : 2 * C].reshape(H, D)
    bv = bqkv[2 * C : 3 * C].reshape(H, D)

    in_maps = []
    for r in range(NCORES):
        ha, hb = 2 * r, 2 * r + 1
        wqk_s = np.ascontiguousarray(
            np.concatenate(
                [Wq[:, ha], Wq[:, hb], Wk[:, ha], Wk[:, hb]], axis=1
            ).astype(ml_dtypes.bfloat16)
        )
        bqk_s = np.ascontiguousarray(
            np.stack([bq[ha], bq[hb], bk[ha], bk[hb]], axis=1)
        )  # [128, 4]
        wv_s = np.ascontiguousarray(
            np.concatenate([Wv[:, ha], Wv[:, hb]], axis=1).astype(
                ml_dtypes.bfloat16
            )
        )
        bv_s = np.ascontiguousarray(
            np.broadcast_to(
                np.concatenate([bv[ha], bv[hb]])[None, :], (128, FV)
            )
        )
        in_maps.append(
            {
                "xT": xT,
                "wqk": wqk_s,
                "wv": wv_s,
                "bqk": bqk_s,
                "bv": bv_s,
                "wproj": Wproj,
                "bproj": bproj_b,
                "cosd": cosT,
                "sind": sinT,
                "rmat": rmat,
            }
        )
    return in_maps


def kernel(x, Wqkv, bqkv, Wproj, bproj, _trace=False, _trace_kwargs=None):
    if "nc" not in _CACHE:
        _CACHE["nc"] = _build_program()
    nc = _CACHE["nc"]
    in_maps = _prep_inputs(x, Wqkv, bqkv, Wproj, bproj)
    kwargs = {}
    if _trace:
        kwargs.update(trace=True, **(_trace_kwargs or {}))
    res = run_bass_kernel_spmd(nc, in_maps, core_ids=list(range(NCORES)), **kwargs)
    _CACHE["last_results"] = res
    # core r's out: [4*128, C]; row block b = tokens [b*T + r*128, +128)
    full = np.stack(
        [res.results[r]["out"].reshape(B, STRIP, C) for r in range(NCORES)],
        axis=1,
    )  # [B, NCORES, STRIP, C]
    return np.ascontiguousarray(
        full.reshape(B, T, C).astype(np.float32)
    )


# revision 6
# speedup vs baseline: 1.1057x; 1.1057x over previous
"""Trainium2 Bass kernel for causal self-attention with RoPE (Megatron-style
head-parallel over 8 NeuronCores).

Sharding: 16 heads / 8 cores = 2 heads per core. Wqkv split column-wise by
head; attention embarrassingly parallel over (batch, head); output projection
row-parallel with the partial contraction exchanged via per-batch AllToAlls
(4 collectives, each launched as soon as that batch's attention completes, so
the exchange overlaps later compute). Core r ends up owning a 128-token strip
of each batch: tokens [b*1024 + r*128, b*1024 + (r+1)*128).

All matmuls bf16 with fp32 PSUM accumulation. Softmax skips max-subtraction
(scores are O(+-10) here). The denominator: exp chunks are accumulated on the
DVE, then one all-ones [128,128] matmul produces the partition-broadcast
denominator directly in PSUM (no gpsimd reduction, ~300ns of PE). Score
matmuls and exp are trimmed to the causally-live column range (the diagonal
masks also zero the dead region). RoPE rotate-half runs as a +-1 permutation
matmul on the PE. Emission interleaves attention/projection units between QKV
matmul groups so scalar-engine exp and DVE latency hide under PE streaming.
"""

import sys

if "/opt/trn_rl_repo" not in sys.path:
    sys.path.insert(0, "/opt/trn_rl_repo")

import ml_dtypes
import numpy as np

import concourse.bacc as bacc
import concourse.bass as bass
import concourse.mybir as mybir
import concourse.tile as tile
from concourse.bass_utils import run_bass_kernel_spmd

B, T, C, H, D = 4, 1024, 2048, 16, 128
TQ = B * T
NCORES = 8
HPC = H // NCORES    # heads per core = 2
FQK = 4 * D          # 512 qkT feature rows per core (qa, qb, ka, kb)
FV = HPC * D         # 256 v feature cols per core
STRIP = T // NCORES  # 128 tokens per (core, batch)
NCT = C // 128       # 16 contraction tiles
SCALE = 1.0 / float(np.sqrt(D))

F32 = mybir.dt.float32
BF16 = mybir.dt.bfloat16

_CACHE = {}


def _build_program():
    nc = bacc.Bacc(
        "TRN2",
        target_bir_lowering=False,
        debug=False,
        enable_asserts=False,
        num_devices=NCORES,
    )

    # ---- I/O -----------------------------------------------------------
    xT = nc.dram_tensor("xT", [C, TQ], BF16, kind="ExternalInput")
    wqk = nc.dram_tensor("wqk", [C, FQK], BF16, kind="ExternalInput")
    wv = nc.dram_tensor("wv", [C, FV], BF16, kind="ExternalInput")
    bqk = nc.dram_tensor("bqk", [128, 4], F32, kind="ExternalInput")
    bv = nc.dram_tensor("bv", [128, FV], F32, kind="ExternalInput")
    wproj = nc.dram_tensor("wproj", [C, C], BF16, kind="ExternalInput")
    bproj = nc.dram_tensor("bproj", [128, C], F32, kind="ExternalInput")
    cosd = nc.dram_tensor("cosd", [128, T], BF16, kind="ExternalInput")
    sind = nc.dram_tensor("sind", [128, T], BF16, kind="ExternalInput")
    rmat = nc.dram_tensor("rmat", [128, 128], BF16, kind="ExternalInput")
    out = nc.dram_tensor("out", [B * STRIP, C], F32, kind="ExternalOutput")

    Exp = mybir.ActivationFunctionType.Exp
    add = mybir.AluOpType.add
    mult = mybir.AluOpType.mult

    with tile.TileContext(nc) as tc:
        with (
            tc.tile_pool(name="const", bufs=1) as cpool,
            tc.tile_pool(name="act", bufs=2) as act,
            tc.tile_pool(name="work", bufs=2) as wpool,
            tc.tile_pool(name="att", bufs=2) as apool,
            tc.tile_pool(name="psA", bufs=2, space="PSUM") as psA,
            tc.tile_pool(name="psB", bufs=2, space="PSUM") as psB,
            tc.tile_pool(name="dram", bufs=1, space="DRAM") as dpool,
        ):
            # ---- startup loads, most-urgent first ----------------------
            # wqk on gpsimd + x chunk 0 on sync, interleaved fine pieces.
            wqk_sb = cpool.tile([128, NCT, FQK], BF16)
            wqk_r = wqk.rearrange("(ct p) f -> p ct f", p=128)
            xt_tiles = {}
            xt_tiles[0] = wpool.tile(
                [128, NCT, 512], BF16, tag="xT_ch", name="xT_ch0"
            )
            xT_r0 = xT[:, 0:512].rearrange("(ct p) t -> p ct t", p=128)
            for pc in range(4):
                s = slice(pc * 4, (pc + 1) * 4)
                nc.gpsimd.dma_start(out=wqk_sb[:, s, :], in_=wqk_r[:, s, :])
                nc.sync.dma_start(out=xt_tiles[0][:, s, :], in_=xT_r0[:, s, :])

            # rope constants (needed ~8us in) on scalar queue
            bqk_sb = cpool.tile([128, 4], F32)
            nc.scalar.dma_start(out=bqk_sb[:], in_=bqk[:])
            cos_sb = cpool.tile([128, T], BF16)
            nc.scalar.dma_start(out=cos_sb[:], in_=cosd[:])
            sin_sb = cpool.tile([128, T], BF16)
            nc.scalar.dma_start(out=sin_sb[:], in_=sind[:])
            rmat_sb = cpool.tile([128, 128], BF16)
            nc.scalar.dma_start(out=rmat_sb[:], in_=rmat[:])

            # x chunk 1 next on sync
            xt_tiles[1] = wpool.tile(
                [128, NCT, 512], BF16, tag="xT_ch", name="xT_ch1"
            )
            xT_r1 = xT[:, 512:1024].rearrange("(ct p) t -> p ct t", p=128)
            for pc in range(2):
                s = slice(pc * 8, (pc + 1) * 8)
                nc.sync.dma_start(out=xt_tiles[1][:, s, :], in_=xT_r1[:, s, :])

            # wv on gpsimd (needed ~20us in)
            wv_sb = cpool.tile([128, NCT, FV], BF16)
            wv_r = wv.rearrange("(ct p) f -> p ct f", p=128)
            for pc in range(2):
                s = slice(pc * 8, (pc + 1) * 8)
                nc.gpsimd.dma_start(out=wv_sb[:, s, :], in_=wv_r[:, s, :])

            bv_sb = cpool.tile([128, FV], F32)
            nc.scalar.dma_start(out=bv_sb[:], in_=bv[:])
            bproj_sb = cpool.tile([128, C], F32)
            nc.scalar.dma_start(out=bproj_sb[:], in_=bproj[:])

            # attention constants on gpsimd
            ones_sb = cpool.tile([128, 128], BF16)
            nc.gpsimd.memset(ones_sb[:], 1.0)
            # diagonal-block masks: mask_m[p, col] = 1 if col >= p + 128*m
            mask_sb = cpool.tile([128, 4, 512], BF16)
            nc.gpsimd.memset(mask_sb[:], 1.0)
            for m in range(4):
                nc.gpsimd.affine_select(
                    out=mask_sb[:, m, :],
                    in_=mask_sb[:, m, :],
                    compare_op=mybir.AluOpType.is_ge,
                    fill=0.0,
                    base=-128 * m,
                    pattern=[[1, 512]],
                    channel_multiplier=-1,
                )
            # one-time zero of the 3 rotating exp-output buffers so stale
            # data in causally-trimmed (never-written) columns is finite;
            # the diagonal masks multiplicatively zero those columns.
            for i in range(3):
                t_ = apool.tile(
                    [128, 2, 512], BF16, tag="pt", bufs=3, name=f"ptz{i}"
                )
                nc.gpsimd.memset(t_[:], 0.0)

            # Wproj chunks: all 4 resident; big loads on gpsimd (idle
            # until the first collective), done well before proj(0).
            wp_tiles = {}
            for ec in range(4):
                wpt = cpool.tile([128, NCT, 512], BF16, name=f"wp{ec}")
                nc.gpsimd.dma_start(
                    out=wpt[:],
                    in_=wproj[:, ec * 512 : (ec + 1) * 512].rearrange(
                        "(ft p) e -> p ft e", p=128
                    ),
                )
                wp_tiles[ec] = wpt

            # a2a buffers: per batch, slot p carries my 2 heads' yT for
            # token strip p of that batch (128 tokens).
            a2a_in = [
                dpool.tile([NCORES, FV, STRIP], BF16, name=f"a2a_in{b}")
                for b in range(B)
            ]
            a2a_out = [
                dpool.tile([NCORES, FV, STRIP], BF16, name=f"a2a_out{b}")
                for b in range(B)
            ]

            qk_tiles = {}
            v_tiles = {}
            yts_tiles = {}

            # ---- QKV projection + RoPE: 16 emission units per batch ---
            def qkv_units(b):
                qkT_b = act.tile(
                    [128, 4, T], BF16, tag="qkT", name=f"qkT{b}"
                )
                v_b = act.tile(
                    [128, T // 128, FV], BF16, tag="vsb", name=f"v{b}"
                )
                qk_tiles[b] = qkT_b
                v_tiles[b] = v_b
                units = []
                for cc in range(2):
                    ch = 2 * b + cc
                    t0 = cc * 512

                    def mk_load(ch=ch):
                        def emit():
                            if ch in xt_tiles:
                                return xt_tiles[ch]
                            xT_ch = wpool.tile(
                                [128, NCT, 512], BF16, tag="xT_ch",
                                name=f"xT_ch{ch}",
                            )
                            xt_tiles[ch] = xT_ch
                            g0 = ch * 512
                            xT_r = xT[:, g0 : g0 + 512].rearrange(
                                "(ct p) t -> p ct t", p=128
                            )
                            for pc in range(2):
                                s = slice(pc * 8, (pc + 1) * 8)
                                nc.sync.dma_start(
                                    out=xT_ch[:, s, :], in_=xT_r[:, s, :]
                                )
                            return xT_ch
                        return emit

                    load = mk_load()

                    def mk_mi(mi, load=load, t0=t0, qkT_b=qkT_b):
                        def emit():
                            xT_ch = load()
                            ps = psA.tile([128, 2, 512], F32, tag="mm512")
                            for ct in range(NCT):
                                nc.tensor.matmul(
                                    ps[:, 0, :],
                                    lhsT=wqk_sb[
                                        :, ct, mi * 128 : (mi + 1) * 128
                                    ],
                                    rhs=xT_ch[:, ct, :],
                                    start=(ct == 0),
                                    stop=(ct == NCT - 1),
                                )
                            # bias + RoPE: dst = (ps+b)*cos + R^T@((ps+b)*sin)
                            m1 = wpool.tile([128, 512], BF16, tag="rope_m1")
                            m2 = wpool.tile([128, 512], BF16, tag="rope_m2")
                            nc.vector.scalar_tensor_tensor(
                                out=m2[:], in0=ps[:, 0, :],
                                scalar=bqk_sb[:, mi : mi + 1],
                                in1=sin_sb[:, t0 : t0 + 512],
                                op0=add, op1=mult,
                            )
                            rot_ps = psB.tile([128, 512], F32, tag="aux")
                            nc.tensor.matmul(
                                rot_ps[:], lhsT=rmat_sb[:], rhs=m2[:],
                                start=True, stop=True,
                            )
                            nc.vector.scalar_tensor_tensor(
                                out=m1[:], in0=ps[:, 0, :],
                                scalar=bqk_sb[:, mi : mi + 1],
                                in1=cos_sb[:, t0 : t0 + 512],
                                op0=add, op1=mult,
                            )
                            nc.vector.tensor_add(
                                qkT_b[:, mi, t0 : t0 + 512], m1[:], rot_ps[:]
                            )
                        return emit

                    def mk_tt(tt, load=load, cc=cc, v_b=v_b):
                        def emit():
                            xT_ch = load()
                            psv = psB.tile([128, FV], F32, tag="acc")
                            for ct in range(NCT):
                                nc.tensor.matmul(
                                    psv[:],
                                    lhsT=xT_ch[
                                        :, ct, tt * 128 : (tt + 1) * 128
                                    ],
                                    rhs=wv_sb[:, ct, :],
                                    start=(ct == 0),
                                    stop=(ct == NCT - 1),
                                )
                            nc.vector.tensor_add(
                                v_b[:, cc * 4 + tt, :], psv[:], bv_sb[:]
                            )
                        return emit

                    for mi in range(4):
                        units.append(mk_mi(mi))
                    for tt in range(4):
                        units.append(mk_tt(tt))
                return units

            # ---- attention: 4 units per batch (hl, tqc); the last unit
            # issues the collective and the yts gather for proj ---------
            def att_units(b):
                def mk_unit(hl, tqc, last):
                    def emit():
                        qkT_b = qk_tiles[b]
                        v_b = v_tiles[b]
                        qh = qkT_b[:, hl, :]
                        kh = qkT_b[:, 2 + hl, :]
                        q0 = tqc * 512
                        nj = 4 * (tqc + 1)
                        ot_ps = psB.tile([128, 512], F32, tag="acc")
                        sacc = apool.tile([128, 512], F32, tag="sacc")
                        sacc_bf = apool.tile([128, 512], BF16, tag="saccb")
                        nadds = 0
                        for jp in range(nj // 2):
                            js = [2 * jp, 2 * jp + 1]
                            md = min(
                                max(0, 128 * (j - (nj - 4))) for j in js
                            )
                            st_ps = psA.tile(
                                [128, 2, 512], F32, tag="mm512"
                            )
                            for jj, j in enumerate(js):
                                s0 = j * 128
                                nc.tensor.matmul(
                                    st_ps[:, jj, md:512],
                                    lhsT=kh[:, s0 : s0 + 128],
                                    rhs=qh[:, q0 + md : q0 + 512],
                                    start=True,
                                    stop=True,
                                )
                            ptp = apool.tile(
                                [128, 2, 512], BF16, tag="pt", bufs=3
                            )
                            nc.scalar.activation(
                                ptp[:, :, md:512], st_ps[:, :, md:512],
                                Exp, scale=SCALE,
                            )
                            for jj, j in enumerate(js):
                                m = j - (nj - 4)
                                if m >= 0:
                                    nc.vector.tensor_mul(
                                        ptp[:, jj, :], ptp[:, jj, :],
                                        mask_sb[:, m, :],
                                    )
                            # DVE accumulation of exp chunks; the final
                            # add writes bf16 for the ones-matmul.
                            for jj in range(2):
                                nadds += 1
                                if nadds == 1:
                                    continue
                                if nadds == 2:
                                    nc.vector.tensor_add(
                                        sacc[:], ptp[:, 0, :], ptp[:, 1, :]
                                    )
                                elif nadds == nj:
                                    nc.vector.tensor_add(
                                        sacc_bf[:], sacc[:], ptp[:, jj, :]
                                    )
                                else:
                                    nc.vector.tensor_add(
                                        sacc[:], sacc[:], ptp[:, jj, :]
                                    )
                            for jj, j in enumerate(js):
                                vt = v_b[:, j, hl * 128 : (hl + 1) * 128]
                                nc.tensor.matmul(
                                    ot_ps[:], lhsT=vt, rhs=ptp[:, jj, :],
                                    start=(j == 0), stop=(j == nj - 1),
                                )
                        # partition-broadcast denominator in one matmul
                        den_ps = psB.tile([128, 512], F32, tag="aux")
                        nc.tensor.matmul(
                            den_ps[:], lhsT=ones_sb[:], rhs=sacc_bf[:],
                            start=True, stop=True,
                        )
                        recipb = apool.tile([128, 512], F32, tag="recipb")
                        nc.vector.reciprocal_approx_fast(recipb[:], den_ps[:])
                        yt = apool.tile([128, 512], BF16, tag="yt")
                        nc.vector.tensor_mul(yt[:], ot_ps[:], recipb[:])
                        # strips of 128 tokens -> a2a slots (one DMA)
                        dst = a2a_in[b][
                            tqc * 4 : (tqc + 1) * 4,
                            hl * 128 : (hl + 1) * 128,
                            :,
                        ].rearrange("s d t -> d s t")
                        nc.scalar.dma_start(
                            out=dst,
                            in_=yt[:].rearrange("d (s t) -> d s t", s=4),
                        )
                        if last:
                            nc.gpsimd.collective_compute(
                                "AllToAll",
                                mybir.AluOpType.bypass,
                                replica_groups=[list(range(NCORES))],
                                ins=[a2a_in[b][:].opt()],
                                outs=[a2a_out[b][:].opt()],
                            )
                            # gather my strip immediately (gpsimd queue is
                            # idle; later-emitted collectives would raise
                            # this DMA's wait threshold)
                            yts = wpool.tile(
                                [128, NCT, STRIP], BF16, tag="yts",
                                name=f"yts{b}",
                            )
                            yts_tiles[b] = yts
                            nc.gpsimd.dma_start(
                                out=yts[:],
                                in_=a2a_out[b].rearrange(
                                    "g (f2 p) t -> p (g f2) t", p=128
                                ),
                            )
                    return emit

                units = []
                for hl in range(HPC):
                    for tqc in range(2):
                        units.append(
                            mk_unit(hl, tqc, hl == HPC - 1 and tqc == 1)
                        )
                return units

            # ---- projection of my 128-token strip: 4 units per batch --
            def proj_units(b):
                def mk_ec(ec):
                    def emit():
                        yts = yts_tiles[b]
                        e0 = ec * 512
                        pps = psA.tile([128, 2, 512], F32, tag="mm512")
                        for ft in range(NCT):
                            nc.tensor.matmul(
                                pps[:, 0, :],
                                lhsT=yts[:, ft, :],
                                rhs=wp_tiles[ec][:, ft, :],
                                start=(ft == 0),
                                stop=(ft == NCT - 1),
                            )
                        osb = wpool.tile([128, 512], F32, tag="osb")
                        nc.vector.tensor_add(
                            osb[:], pps[:, 0, :], bproj_sb[:, e0 : e0 + 512]
                        )
                        nc.sync.dma_start(
                            out=out[
                                b * STRIP : (b + 1) * STRIP, e0 : e0 + 512
                            ],
                            in_=osb[:],
                        )
                    return emit

                return [mk_ec(ec) for ec in range(4)]

            # ---- schedule: interleave fillers 1-per-2 qkv groups ------
            def interleave(groups, fillers):
                fi = 0
                for gi, g in enumerate(groups):
                    g()
                    if gi % 2 == 1 and fi < len(fillers):
                        fillers[fi]()
                        fi += 1
                for f in fillers[fi:]:
                    f()

            interleave(qkv_units(0), [])
            interleave(qkv_units(1), att_units(0))
            interleave(qkv_units(2), att_units(1) + proj_units(0))
            interleave(qkv_units(3), att_units(2) + proj_units(1))
            interleave([], att_units(3) + proj_units(2))
            interleave([], proj_units(3))

    nc.compile()
    return nc


def _rope_tables():
    inv = 1.0 / (10000.0 ** (np.arange(0, D, 2, dtype=np.float64) / D))
    t = np.arange(T, dtype=np.float64)
    fr = np.outer(t, inv)  # [T, 64]
    cosT = np.tile(np.cos(fr).T, (2, 1)).astype(ml_dtypes.bfloat16)
    sinT = np.tile(np.sin(fr).T, (2, 1)).astype(ml_dtypes.bfloat16)
    return np.ascontiguousarray(cosT), np.ascontiguousarray(sinT)


def _prep_inputs(x, Wqkv, bqkv, Wproj, bproj):
    x = np.asarray(x, np.float32).reshape(TQ, C)
    Wqkv = np.asarray(Wqkv, np.float32)
    bqkv = np.asarray(bqkv, np.float32)
    Wproj = np.ascontiguousarray(
        np.asarray(Wproj, np.float32).astype(ml_dtypes.bfloat16)
    )
    bproj = np.asarray(bproj, np.float32)

    xT = np.ascontiguousarray(x.T.astype(ml_dtypes.bfloat16))
    cosT, sinT = _rope_tables()
    rmat = np.zeros((128, 128), ml_dtypes.bfloat16)
    for i in range(64):
        rmat[64 + i, i] = -1.0   # out[p<64]  = -m2[p+64]
        rmat[i, 64 + i] = 1.0    # out[p>=64] = +m2[p-64]
    bproj_b = np.ascontiguousarray(np.broadcast_to(bproj[None, :], (128, C)))

    Wq = Wqkv[:, 0 * C : 1 * C].reshape(C, H, D)
    Wk = Wqkv[:, 1 * C : 2 * C].reshape(C, H, D)
    Wv = Wqkv[:, 2 * C : 3 * C].reshape(C, H, D)
    bq = bqkv[0 * C : 1 * C].reshape(H, D)
    bk = bqkv[1 * C : 2 * C].reshape(H, D)
    bv = bqkv[2 * C : 3 * C].reshape(H, D)

    in_maps = []
    for r in range(NCORES):
        ha, hb = 2 * r, 2 * r + 1
        wqk_s = np.ascontiguousarray(
            np.concatenate(
                [Wq[:, ha], Wq[:, hb], Wk[:, ha], Wk[:, hb]], axis=1
            ).astype(ml_dtypes.bfloat16)
        )
        bqk_s = np.ascontiguousarray(
            np.stack([bq[ha], bq[hb], bk[ha], bk[hb]], axis=1)
        )  # [128, 4]
        wv_s = np.ascontiguousarray(
            np.concatenate([Wv[:, ha], Wv[:, hb]], axis=1).astype(
                ml_dtypes.bfloat16
            )
        )
        bv_s = np.ascontiguousarray(
            np.broadcast_to(
                np.concatenate([bv[ha], bv[hb]])[None, :], (128, FV)
            )
        )
        in_maps.append(
            {
                "xT": xT,
                "wqk": wqk_s,
                "wv": wv_s,
                "bqk": bqk_s,
                "bv": bv_s,
                "wproj": Wproj,
                "bproj": bproj_b,
                "cosd": cosT,
                "sind": sinT,
                "rmat": rmat,
            }
        )
    return in_maps


def kernel(x, Wqkv, bqkv, Wproj, bproj, _trace=False, _trace_kwargs=None):
    if "nc" not in _CACHE:
        _CACHE["nc"] = _build_program()
    nc = _CACHE["nc"]
    in_maps = _prep_inputs(x, Wqkv, bqkv, Wproj, bproj)
    kwargs = {}
    if _trace:
        kwargs.update(trace=True, **(_trace_kwargs or {}))
    res = run_bass_kernel_spmd(nc, in_maps, core_ids=list(range(NCORES)), **kwargs)
    _CACHE["last_results"] = res
    # core r's out: [4*128, C]; row block b = tokens [b*T + r*128, +128)
    full = np.stack(
        [res.results[r]["out"].reshape(B, STRIP, C) for r in range(NCORES)],
        axis=1,
    )  # [B, NCORES, STRIP, C]
    return np.ascontiguousarray(
        full.reshape(B, T, C).astype(np.float32)
    )


# revision 10
# speedup vs baseline: 1.1852x; 1.0719x over previous
"""Trainium2 Bass kernel for causal self-attention with RoPE (Megatron-style
head-parallel over 8 NeuronCores).

Sharding: 16 heads / 8 cores = 2 heads per core. Wqkv split column-wise by
head; attention embarrassingly parallel over (batch, head); output projection
row-parallel with the partial contraction exchanged via per-batch AllToAlls
(4 collectives, each launched as soon as that batch's attention completes, so
the exchange overlaps later compute). Core r ends up owning a 128-token strip
of each batch: tokens [b*1024 + r*128, b*1024 + (r+1)*128).

All matmuls bf16 with fp32 PSUM accumulation. Softmax skips max-subtraction
(scores are O(+-10) here). The denominator: exp chunks are accumulated on the
DVE, then one all-ones [128,128] matmul produces the partition-broadcast
denominator directly in PSUM (no gpsimd reduction, ~300ns of PE). Score
matmuls and exp are trimmed to the causally-live column range (the diagonal
masks also zero the dead region). RoPE rotate-half runs as a +-1 permutation
matmul on the PE. Emission interleaves attention/projection units between QKV
matmul groups so scalar-engine exp and DVE latency hide under PE streaming.
"""

import sys

if "/opt/trn_rl_repo" not in sys.path:
    sys.path.insert(0, "/opt/trn_rl_repo")

import ml_dtypes
import numpy as np

import concourse.bacc as bacc
import concourse.bass as bass
import concourse.mybir as mybir
import concourse.tile as tile
from concourse.bass_utils import run_bass_kernel_spmd

B, T, C, H, D = 4, 1024, 2048, 16, 128
TQ = B * T
NCORES = 8
HPC = H // NCORES    # heads per core = 2
FQK = 4 * D          # 512 qkT feature rows per core (qa, qb, ka, kb)
FV = HPC * D         # 256 v feature cols per core
STRIP = T // NCORES  # 128 tokens per (core, batch)
NCT = C // 128       # 16 contraction tiles
SCALE = 1.0 / float(np.sqrt(D))

F32 = mybir.dt.float32
BF16 = mybir.dt.bfloat16

_CACHE = {}


def _build_program():
    nc = bacc.Bacc(
        "TRN2",
        target_bir_lowering=False,
        debug=False,
        enable_asserts=False,
        num_devices=NCORES,
    )

    # ---- I/O -----------------------------------------------------------
    xT = nc.dram_tensor("xT", [C, TQ], BF16, kind="ExternalInput")
    wqk = nc.dram_tensor("wqk", [C, FQK], BF16, kind="ExternalInput")
    wv = nc.dram_tensor("wv", [C, FV], BF16, kind="ExternalInput")
    bqk = nc.dram_tensor("bqk", [128, 4], F32, kind="ExternalInput")
    bv = nc.dram_tensor("bv", [128, FV], F32, kind="ExternalInput")
    wproj = nc.dram_tensor("wproj", [C, C], BF16, kind="ExternalInput")
    bproj = nc.dram_tensor("bproj", [128, C], F32, kind="ExternalInput")
    cosd = nc.dram_tensor("cosd", [128, T], BF16, kind="ExternalInput")
    sind = nc.dram_tensor("sind", [128, T], BF16, kind="ExternalInput")
    rmat = nc.dram_tensor("rmat", [128, 128], BF16, kind="ExternalInput")
    out = nc.dram_tensor("out", [B * STRIP, C], F32, kind="ExternalOutput")

    Exp = mybir.ActivationFunctionType.Exp
    add = mybir.AluOpType.add
    mult = mybir.AluOpType.mult

    with tile.TileContext(nc) as tc:
        with (
            tc.tile_pool(name="const", bufs=1) as cpool,
            tc.tile_pool(name="act", bufs=2) as act,
            tc.tile_pool(name="work", bufs=2) as wpool,
            tc.tile_pool(name="att", bufs=2) as apool,
            tc.tile_pool(name="psA", bufs=2, space="PSUM") as psA,
            tc.tile_pool(name="psB", bufs=2, space="PSUM") as psB,
            tc.tile_pool(name="dram", bufs=1, space="DRAM") as dpool,
        ):
            # ---- startup loads, most-urgent first ----------------------
            # wqk on gpsimd + x chunk 0 on sync, interleaved fine pieces.
            wqk_sb = cpool.tile([128, NCT, FQK], BF16)
            wqk_r = wqk.rearrange("(ct p) f -> p ct f", p=128)
            xt_tiles = {}
            xt_tiles[0] = wpool.tile(
                [128, NCT, 512], BF16, tag="xT_ch", name="xT_ch0"
            )
            xT_r0 = xT[:, 0:512].rearrange("(ct p) t -> p ct t", p=128)
            for pc in range(4):
                s = slice(pc * 4, (pc + 1) * 4)
                nc.gpsimd.dma_start(out=wqk_sb[:, s, :], in_=wqk_r[:, s, :])
                nc.sync.dma_start(out=xt_tiles[0][:, s, :], in_=xT_r0[:, s, :])

            # rope constants (needed ~8us in) on scalar queue
            bqk_sb = cpool.tile([128, 4], F32)
            nc.scalar.dma_start(out=bqk_sb[:], in_=bqk[:])
            cos_sb = cpool.tile([128, T], BF16)
            nc.scalar.dma_start(out=cos_sb[:], in_=cosd[:])
            sin_sb = cpool.tile([128, T], BF16)
            nc.scalar.dma_start(out=sin_sb[:], in_=sind[:])
            rmat_sb = cpool.tile([128, 128], BF16)
            nc.scalar.dma_start(out=rmat_sb[:], in_=rmat[:])

            # x chunk 1 next on sync
            xt_tiles[1] = wpool.tile(
                [128, NCT, 512], BF16, tag="xT_ch", name="xT_ch1"
            )
            xT_r1 = xT[:, 512:1024].rearrange("(ct p) t -> p ct t", p=128)
            for pc in range(2):
                s = slice(pc * 8, (pc + 1) * 8)
                nc.sync.dma_start(out=xt_tiles[1][:, s, :], in_=xT_r1[:, s, :])

            # wv on gpsimd (needed ~20us in)
            wv_sb = cpool.tile([128, NCT, FV], BF16)
            wv_r = wv.rearrange("(ct p) f -> p ct f", p=128)
            for pc in range(2):
                s = slice(pc * 8, (pc + 1) * 8)
                nc.gpsimd.dma_start(out=wv_sb[:, s, :], in_=wv_r[:, s, :])

            bv_sb = cpool.tile([128, FV], F32)
            nc.scalar.dma_start(out=bv_sb[:], in_=bv[:])
            bproj_sb = cpool.tile([128, C], F32)  # loaded later (filler)

            # attention constants on gpsimd
            ones_sb = cpool.tile([128, 128], BF16)
            nc.gpsimd.memset(ones_sb[:], 1.0)
            # diagonal-block masks: mask_m[p, col] = 1 if col >= p + 128*m
            mask_sb = cpool.tile([128, 4, 512], BF16)
            nc.gpsimd.memset(mask_sb[:], 1.0)
            for m in range(4):
                nc.gpsimd.affine_select(
                    out=mask_sb[:, m, :],
                    in_=mask_sb[:, m, :],
                    compare_op=mybir.AluOpType.is_ge,
                    fill=0.0,
                    base=-128 * m,
                    pattern=[[1, 512]],
                    channel_multiplier=-1,
                )
            # one-time zero of the 3 rotating exp-output buffers so stale
            # data in causally-trimmed (never-written) columns is finite;
            # the diagonal masks multiplicatively zero those columns.
            for i in range(3):
                t_ = apool.tile(
                    [128, 2, 512], BF16, tag="pt", bufs=3, name=f"ptz{i}"
                )
                nc.gpsimd.memset(t_[:], 0.0)

            # Wproj chunks: all 4 resident; loads deferred (emitted as
            # fillers during batch-0 compute) so they don't steal HBM
            # bandwidth from the startup-critical wqk/x loads.
            wp_tiles = {}

            def mk_wp_load(ec):
                def emit():
                    wpt = cpool.tile([128, NCT, 512], BF16, name=f"wp{ec}")
                    nc.gpsimd.dma_start(
                        out=wpt[:],
                        in_=wproj[:, ec * 512 : (ec + 1) * 512].rearrange(
                            "(ft p) e -> p ft e", p=128
                        ),
                    )
                    wp_tiles[ec] = wpt
                return emit

            def mk_bproj_load():
                def emit():
                    nc.gpsimd.dma_start(out=bproj_sb[:], in_=bproj[:])
                return emit

            # a2a buffers: per batch, slot p carries my 2 heads' yT for
            # token strip p of that batch (128 tokens).
            a2a_in = [
                dpool.tile([NCORES, FV, STRIP], BF16, name=f"a2a_in{b}")
                for b in range(B)
            ]
            a2a_out = [
                dpool.tile([NCORES, FV, STRIP], BF16, name=f"a2a_out{b}")
                for b in range(B)
            ]

            qk_tiles = {}
            v_tiles = {}
            yts_tiles = {}

            # ---- QKV projection + RoPE: 16 emission units per batch ---
            def qkv_units(b):
                qkT_b = act.tile(
                    [128, 4, T], BF16, tag="qkT", name=f"qkT{b}"
                )
                v_b = act.tile(
                    [128, T // 128, FV], BF16, tag="vsb", name=f"v{b}"
                )
                qk_tiles[b] = qkT_b
                v_tiles[b] = v_b
                units = []
                for cc in range(2):
                    ch = 2 * b + cc
                    t0 = cc * 512

                    def mk_load(ch=ch):
                        def emit():
                            if ch in xt_tiles:
                                return xt_tiles[ch]
                            xT_ch = wpool.tile(
                                [128, NCT, 512], BF16, tag="xT_ch",
                                name=f"xT_ch{ch}",
                            )
                            xt_tiles[ch] = xT_ch
                            g0 = ch * 512
                            xT_r = xT[:, g0 : g0 + 512].rearrange(
                                "(ct p) t -> p ct t", p=128
                            )
                            for pc in range(2):
                                s = slice(pc * 8, (pc + 1) * 8)
                                nc.sync.dma_start(
                                    out=xT_ch[:, s, :], in_=xT_r[:, s, :]
                                )
                            return xT_ch
                        return emit

                    load = mk_load()

                    def mk_mi(mi, load=load, t0=t0, qkT_b=qkT_b):
                        def emit():
                            xT_ch = load()
                            ps = psA.tile([128, 2, 512], F32, tag="mm512")
                            for ct in range(NCT):
                                nc.tensor.matmul(
                                    ps[:, 0, :],
                                    lhsT=wqk_sb[
                                        :, ct, mi * 128 : (mi + 1) * 128
                                    ],
                                    rhs=xT_ch[:, ct, :],
                                    start=(ct == 0),
                                    stop=(ct == NCT - 1),
                                )
                            # bias + RoPE: dst = (ps+b)*cos + R^T@((ps+b)*sin)
                            m1 = wpool.tile([128, 512], BF16, tag="rope_m1")
                            m2 = wpool.tile([128, 512], BF16, tag="rope_m2")
                            nc.vector.scalar_tensor_tensor(
                                out=m2[:], in0=ps[:, 0, :],
                                scalar=bqk_sb[:, mi : mi + 1],
                                in1=sin_sb[:, t0 : t0 + 512],
                                op0=add, op1=mult,
                            )
                            rot_ps = psB.tile([128, 512], F32, tag="aux")
                            nc.tensor.matmul(
                                rot_ps[:], lhsT=rmat_sb[:], rhs=m2[:],
                                start=True, stop=True,
                            )
                            nc.vector.scalar_tensor_tensor(
                                out=m1[:], in0=ps[:, 0, :],
                                scalar=bqk_sb[:, mi : mi + 1],
                                in1=cos_sb[:, t0 : t0 + 512],
                                op0=add, op1=mult,
                            )
                            nc.vector.tensor_add(
                                qkT_b[:, mi, t0 : t0 + 512], m1[:], rot_ps[:]
                            )
                        return emit

                    def mk_tt(tt, load=load, cc=cc, v_b=v_b):
                        def emit():
                            xT_ch = load()
                            psv = psB.tile([128, FV], F32, tag="acc")
                            for ct in range(NCT):
                                nc.tensor.matmul(
                                    psv[:],
                                    lhsT=xT_ch[
                                        :, ct, tt * 128 : (tt + 1) * 128
                                    ],
                                    rhs=wv_sb[:, ct, :],
                                    start=(ct == 0),
                                    stop=(ct == NCT - 1),
                                )
                            nc.vector.tensor_add(
                                v_b[:, cc * 4 + tt, :], psv[:], bv_sb[:]
                            )
                        return emit

                    for mi in range(4):
                        units.append(mk_mi(mi))
                    for tt in range(4):
                        units.append(mk_tt(tt))
                return units

            # ---- attention: 4 units per batch (hl, tqc); the last unit
            # issues the collective and the yts gather for proj ---------
            def att_units(b):
                def mk_unit(hl, tqc, last):
                    def emit():
                        qkT_b = qk_tiles[b]
                        v_b = v_tiles[b]
                        qh = qkT_b[:, hl, :]
                        kh = qkT_b[:, 2 + hl, :]
                        q0 = tqc * 512
                        nj = 4 * (tqc + 1)
                        ot_ps = psB.tile([128, 512], F32, tag="acc")
                        sacc = apool.tile([128, 512], F32, tag="sacc")
                        sacc_bf = apool.tile([128, 512], BF16, tag="saccb")
                        nadds = 0
                        for jp in range(nj // 2):
                            js = [2 * jp, 2 * jp + 1]
                            md = min(
                                max(0, 128 * (j - (nj - 4))) for j in js
                            )
                            st_ps = psA.tile(
                                [128, 2, 512], F32, tag="mm512"
                            )
                            for jj, j in enumerate(js):
                                s0 = j * 128
                                nc.tensor.matmul(
                                    st_ps[:, jj, md:512],
                                    lhsT=kh[:, s0 : s0 + 128],
                                    rhs=qh[:, q0 + md : q0 + 512],
                                    start=True,
                                    stop=True,
                                )
                            ptp = apool.tile(
                                [128, 2, 512], BF16, tag="pt", bufs=3
                            )
                            nc.scalar.activation(
                                ptp[:, :, md:512], st_ps[:, :, md:512],
                                Exp, scale=SCALE,
                            )
                            for jj, j in enumerate(js):
                                m = j - (nj - 4)
                                if m >= 0:
                                    nc.vector.tensor_mul(
                                        ptp[:, jj, :], ptp[:, jj, :],
                                        mask_sb[:, m, :],
                                    )
                            # DVE accumulation of exp chunks; the final
                            # add writes bf16 for the ones-matmul.
                            for jj in range(2):
                                nadds += 1
                                if nadds == 1:
                                    continue
                                if nadds == 2:
                                    nc.vector.tensor_add(
                                        sacc[:], ptp[:, 0, :], ptp[:, 1, :]
                                    )
                                elif nadds == nj:
                                    nc.vector.tensor_add(
                                        sacc_bf[:], sacc[:], ptp[:, jj, :]
                                    )
                                else:
                                    nc.vector.tensor_add(
                                        sacc[:], sacc[:], ptp[:, jj, :]
                                    )
                            for jj, j in enumerate(js):
                                vt = v_b[:, j, hl * 128 : (hl + 1) * 128]
                                nc.tensor.matmul(
                                    ot_ps[:], lhsT=vt, rhs=ptp[:, jj, :],
                                    start=(j == 0), stop=(j == nj - 1),
                                )
                        # partition-broadcast denominator in one matmul
                        den_ps = psB.tile([128, 512], F32, tag="aux")
                        nc.tensor.matmul(
                            den_ps[:], lhsT=ones_sb[:], rhs=sacc_bf[:],
                            start=True, stop=True,
                        )
                        recipb = apool.tile([128, 512], F32, tag="recipb")
                        nc.vector.reciprocal_approx_fast(recipb[:], den_ps[:])
                        yt = apool.tile([128, 512], BF16, tag="yt")
                        nc.vector.tensor_mul(yt[:], ot_ps[:], recipb[:])
                        # strips of 128 tokens -> a2a slots (one DMA)
                        dst = a2a_in[b][
                            tqc * 4 : (tqc + 1) * 4,
                            hl * 128 : (hl + 1) * 128,
                            :,
                        ].rearrange("s d t -> d s t")
                        nc.scalar.dma_start(
                            out=dst,
                            in_=yt[:].rearrange("d (s t) -> d s t", s=4),
                        )
                        if last:
                            nc.gpsimd.collective_compute(
                                "AllToAll",
                                mybir.AluOpType.bypass,
                                replica_groups=[list(range(NCORES))],
                                ins=[a2a_in[b][:].opt()],
                                outs=[a2a_out[b][:].opt()],
                            )
                            # gather my strip immediately (gpsimd queue is
                            # idle; later-emitted collectives would raise
                            # this DMA's wait threshold)
                            yts = wpool.tile(
                                [128, NCT, STRIP], BF16, tag="yts",
                                name=f"yts{b}", bufs=4,
                            )
                            yts_tiles[b] = yts
                            nc.gpsimd.dma_start(
                                out=yts[:],
                                in_=a2a_out[b].rearrange(
                                    "g (f2 p) t -> p (g f2) t", p=128
                                ),
                            )
                    return emit

                units = []
                for hl in range(HPC):
                    for tqc in range(2):
                        units.append(
                            mk_unit(hl, tqc, hl == HPC - 1 and tqc == 1)
                        )
                return units

            # ---- projection of my 128-token strip: 4 units per batch --
            def proj_units(b):
                def mk_ec(ec):
                    def emit():
                        yts = yts_tiles[b]
                        e0 = ec * 512
                        pps = psA.tile([128, 2, 512], F32, tag="mm512")
                        for ft in range(NCT):
                            nc.tensor.matmul(
                                pps[:, 0, :],
                                lhsT=yts[:, ft, :],
                                rhs=wp_tiles[ec][:, ft, :],
                                start=(ft == 0),
                                stop=(ft == NCT - 1),
                            )
                        osb = wpool.tile([128, 512], F32, tag="osb")
                        nc.vector.tensor_add(
                            osb[:], pps[:, 0, :], bproj_sb[:, e0 : e0 + 512]
                        )
                        nc.sync.dma_start(
                            out=out[
                                b * STRIP : (b + 1) * STRIP, e0 : e0 + 512
                            ],
                            in_=osb[:],
                        )
                    return emit

                return [mk_ec(ec) for ec in range(4)]

            # ---- schedule: interleave fillers 1-per-2 qkv groups ------
            def interleave(groups, fillers):
                fi = 0
                for gi, g in enumerate(groups):
                    g()
                    if gi % 2 == 1 and fi < len(fillers):
                        fillers[fi]()
                        fi += 1
                for f in fillers[fi:]:
                    f()

            # all proj work is deferred to the tail so ~39us of PE work
            # is queued behind the last collective's doorbell, hiding its
            # ~38us latency.
            wp_loads = [mk_wp_load(ec) for ec in range(4)] + [mk_bproj_load()]
            interleave(qkv_units(0), [])
            interleave(qkv_units(1), att_units(0) + wp_loads)
            interleave(qkv_units(2), att_units(1))
            interleave(qkv_units(3), att_units(2))
            a3 = att_units(3)
            p0 = proj_units(0)
            tail = [
                a3[0], p0[0], a3[1], p0[1], a3[2], p0[2], a3[3], p0[3],
            ] + proj_units(1) + proj_units(2) + proj_units(3)
            interleave([], tail)

    nc.compile()
    return nc


def _rope_tables():
    inv = 1.0 / (10000.0 ** (np.arange(0, D, 2, dtype=np.float64) / D))
    t = np.arange(T, dtype=np.float64)
    fr = np.outer(t, inv)  # [T, 64]
    cosT = np.tile(np.cos(fr).T, (2, 1)).astype(ml_dtypes.bfloat16)
    sinT = np.tile(np.sin(fr).T, (2, 1)).astype(ml_dtypes.bfloat16)
    return np.ascontiguousarray(cosT), np.ascontiguousarray(sinT)


def _prep_inputs(x, Wqkv, bqkv, Wproj, bproj):
    x = np.asarray(x, np.float32).reshape(TQ, C)
    Wqkv = np.asarray(Wqkv, np.float32)
    bqkv = np.asarray(bqkv, np.float32)
    Wproj = np.ascontiguousarray(
        np.asarray(Wproj, np.float32).astype(ml_dtypes.bfloat16)
    )
    bproj = np.asarray(bproj, np.float32)

    xT = np.ascontiguousarray(x.T.astype(ml_dtypes.bfloat16))
    cosT, sinT = _rope_tables()
    rmat = np.zeros((128, 128), ml_dtypes.bfloat16)
    for i in range(64):
        rmat[64 + i, i] = -1.0   # out[p<64]  = -m2[p+64]
        rmat[i, 64 + i] = 1.0    # out[p>=64] = +m2[p-64]
    bproj_b = np.ascontiguousarray(np.broadcast_to(bproj[None, :], (128, C)))

    Wq = Wqkv[:, 0 * C : 1 * C].reshape(C, H, D)
    Wk = Wqkv[:, 1 * C : 2 * C].reshape(C, H, D)
    Wv = Wqkv[:, 2 * C : 3 * C].reshape(C, H, D)
    bq = bqkv[0 * C : 1 * C].reshape(H, D)
    bk = bqkv[1 * C : 2 * C].reshape(H, D)
    bv = bqkv[2 * C : 3 * C].reshape(H, D)

    in_maps = []
    for r in range(NCORES):
        ha, hb = 2 * r, 2 * r + 1
        wqk_s = np.ascontiguousarray(
            np.concatenate(
                [Wq[:, ha], Wq[:, hb], Wk[:, ha], Wk[:, hb]], axis=1
            ).astype(ml_dtypes.bfloat16)
        )
        bqk_s = np.ascontiguousarray(
            np.stack([bq[ha], bq[hb], bk[ha], bk[hb]], axis=1)
        )  # [128, 4]
        wv_s = np.ascontiguousarray(
            np.concatenate([Wv[:, ha], Wv[:, hb]], axis=1).astype(
                ml_dtypes.bfloat16
            )
        )
        bv_s = np.ascontiguousarray(
            np.broadcast_to(
                np.concatenate([bv[ha], bv[hb]])[None, :], (128, FV)
            )
        )
        in_maps.append(
            {
                "xT": xT,
                "wqk": wqk_s,
                "wv": wv_s,
                "bqk": bqk_s,
                "bv": bv_s,
                "wproj": Wproj,
                "bproj": bproj_b,
                "cosd": cosT,
                "sind": sinT,
                "rmat": rmat,
            }
        )
    return in_maps


def kernel(x, Wqkv, bqkv, Wproj, bproj, _trace=False, _trace_kwargs=None):
    if "nc" not in _CACHE:
        _CACHE["nc"] = _build_program()
    nc = _CACHE["nc"]
    in_maps = _prep_inputs(x, Wqkv, bqkv, Wproj, bproj)
    kwargs = {}
    if _trace:
        kwargs.update(trace=True, **(_trace_kwargs or {}))
    res = run_bass_kernel_spmd(nc, in_maps, core_ids=list(range(NCORES)), **kwargs)
    _CACHE["last_results"] = res
    # core r's out: [4*128, C]; row block b = tokens [b*T + r*128, +128)
    full = np.stack(
        [res.results[r]["out"].reshape(B, STRIP, C) for r in range(NCORES)],
        axis=1,
    )  # [B, NCORES, STRIP, C]
    return np.ascontiguousarray(
        full.reshape(B, T, C).astype(np.float32)
    )


# revision 11
# speedup vs baseline: 1.2440x; 1.0496x over previous
"""Trainium2 Bass kernel for causal self-attention with RoPE (Megatron-style
head-parallel over 8 NeuronCores).

Sharding: 16 heads / 8 cores = 2 heads per core. Wqkv split column-wise by
head; attention embarrassingly parallel over (batch, head); output projection
row-parallel with the partial contraction exchanged via per-batch AllToAlls
(4 collectives, each launched as soon as that batch's attention completes, so
the exchange overlaps later compute). Core r ends up owning a 128-token strip
of each batch: tokens [b*1024 + r*128, b*1024 + (r+1)*128).

All matmuls bf16 with fp32 PSUM accumulation. Softmax skips max-subtraction
(scores are O(+-10) here). The denominator: exp chunks are accumulated on the
DVE, then one all-ones [128,128] matmul produces the partition-broadcast
denominator directly in PSUM (no gpsimd reduction, ~300ns of PE). Score
matmuls and exp are trimmed to the causally-live column range (the diagonal
masks also zero the dead region). RoPE rotate-half runs as a +-1 permutation
matmul on the PE. Emission interleaves attention/projection units between QKV
matmul groups so scalar-engine exp and DVE latency hide under PE streaming.
"""

import sys

if "/opt/trn_rl_repo" not in sys.path:
    sys.path.insert(0, "/opt/trn_rl_repo")

import ml_dtypes
import numpy as np

import concourse.bacc as bacc
import concourse.bass as bass
import concourse.mybir as mybir
import concourse.tile as tile
from concourse.bass_utils import run_bass_kernel_spmd

B, T, C, H, D = 4, 1024, 2048, 16, 128
TQ = B * T
NCORES = 8
HPC = H // NCORES    # heads per core = 2
FQK = 4 * D          # 512 qkT feature rows per core (qa, qb, ka, kb)
FV = HPC * D         # 256 v feature cols per core
STRIP = T // NCORES  # 128 tokens per (core, batch)
NCT = C // 128       # 16 contraction tiles
SCALE = 1.0 / float(np.sqrt(D))

F32 = mybir.dt.float32
BF16 = mybir.dt.bfloat16

_CACHE = {}


def _build_program():
    nc = bacc.Bacc(
        "TRN2",
        target_bir_lowering=False,
        debug=False,
        enable_asserts=False,
        num_devices=NCORES,
    )

    # ---- I/O -----------------------------------------------------------
    xT = nc.dram_tensor("xT", [C, TQ], BF16, kind="ExternalInput")
    wqk = nc.dram_tensor("wqk", [C, FQK], BF16, kind="ExternalInput")
    wv = nc.dram_tensor("wv", [C, FV], BF16, kind="ExternalInput")
    bqk = nc.dram_tensor("bqk", [128, 4], F32, kind="ExternalInput")
    bv = nc.dram_tensor("bv", [128, FV], F32, kind="ExternalInput")
    wproj = nc.dram_tensor("wproj", [C, C], BF16, kind="ExternalInput")
    bproj = nc.dram_tensor("bproj", [128, C], F32, kind="ExternalInput")
    cosd = nc.dram_tensor("cosd", [128, T], BF16, kind="ExternalInput")
    sind = nc.dram_tensor("sind", [128, T], BF16, kind="ExternalInput")
    rmat = nc.dram_tensor("rmat", [128, 128], BF16, kind="ExternalInput")
    out = nc.dram_tensor("out", [B * STRIP, C], F32, kind="ExternalOutput")

    Exp = mybir.ActivationFunctionType.Exp
    add = mybir.AluOpType.add
    mult = mybir.AluOpType.mult

    with tile.TileContext(nc) as tc:
        with (
            tc.tile_pool(name="const", bufs=1) as cpool,
            tc.tile_pool(name="act", bufs=2) as act,
            tc.tile_pool(name="work", bufs=2) as wpool,
            tc.tile_pool(name="att", bufs=2) as apool,
            tc.tile_pool(name="psA", bufs=2, space="PSUM") as psA,
            tc.tile_pool(name="psB", bufs=2, space="PSUM") as psB,
            tc.tile_pool(name="dram", bufs=1, space="DRAM") as dpool,
        ):
            # ---- startup loads, most-urgent first ----------------------
            # wqk on gpsimd + x chunk 0 on sync, interleaved fine pieces.
            wqk_sb = cpool.tile([128, NCT, FQK], BF16)
            wqk_r = wqk.rearrange("(ct p) f -> p ct f", p=128)
            xt_tiles = {}
            xt_tiles[0] = wpool.tile(
                [128, NCT, 512], BF16, tag="xT_ch", name="xT_ch0"
            )
            xT_r0 = xT[:, 0:512].rearrange("(ct p) t -> p ct t", p=128)
            for pc in range(4):
                s = slice(pc * 4, (pc + 1) * 4)
                nc.gpsimd.dma_start(out=wqk_sb[:, s, :], in_=wqk_r[:, s, :])
                nc.gpsimd.dma_start(out=xt_tiles[0][:, s, :], in_=xT_r0[:, s, :])

            # rope constants (needed ~8us in) on scalar queue
            bqk_sb = cpool.tile([128, 4], F32)
            nc.scalar.dma_start(out=bqk_sb[:], in_=bqk[:])
            cos_sb = cpool.tile([128, T], BF16)
            nc.scalar.dma_start(out=cos_sb[:], in_=cosd[:])
            sin_sb = cpool.tile([128, T], BF16)
            nc.scalar.dma_start(out=sin_sb[:], in_=sind[:])
            rmat_sb = cpool.tile([128, 128], BF16)
            nc.scalar.dma_start(out=rmat_sb[:], in_=rmat[:])

            # x chunk 1 next on sync
            xt_tiles[1] = wpool.tile(
                [128, NCT, 512], BF16, tag="xT_ch", name="xT_ch1"
            )
            xT_r1 = xT[:, 512:1024].rearrange("(ct p) t -> p ct t", p=128)
            for pc in range(2):
                s = slice(pc * 8, (pc + 1) * 8)
                nc.gpsimd.dma_start(out=xt_tiles[1][:, s, :], in_=xT_r1[:, s, :])

            # wv on gpsimd (needed ~20us in)
            wv_sb = cpool.tile([128, NCT, FV], BF16)
            wv_r = wv.rearrange("(ct p) f -> p ct f", p=128)
            for pc in range(2):
                s = slice(pc * 8, (pc + 1) * 8)
                nc.gpsimd.dma_start(out=wv_sb[:, s, :], in_=wv_r[:, s, :])

            bv_sb = cpool.tile([128, FV], F32)
            nc.scalar.dma_start(out=bv_sb[:], in_=bv[:])
            bproj_sb = cpool.tile([128, C], F32)  # loaded later (filler)

            # attention constants on gpsimd
            ones_sb = cpool.tile([128, 128], BF16)
            nc.gpsimd.memset(ones_sb[:], 1.0)
            # diagonal-block masks: mask_m[p, col] = 1 if col >= p + 128*m
            mask_sb = cpool.tile([128, 4, 512], BF16)
            nc.gpsimd.memset(mask_sb[:], 1.0)
            for m in range(4):
                nc.gpsimd.affine_select(
                    out=mask_sb[:, m, :],
                    in_=mask_sb[:, m, :],
                    compare_op=mybir.AluOpType.is_ge,
                    fill=0.0,
                    base=-128 * m,
                    pattern=[[1, 512]],
                    channel_multiplier=-1,
                )
            # one-time zero of the 3 rotating exp-output buffers so stale
            # data in causally-trimmed (never-written) columns is finite;
            # the diagonal masks multiplicatively zero those columns.
            for i in range(3):
                t_ = apool.tile(
                    [128, 2, 512], BF16, tag="pt", bufs=3, name=f"ptz{i}"
                )
                nc.gpsimd.memset(t_[:], 0.0)

            # Wproj chunks: all 4 resident; loads deferred (emitted as
            # fillers during batch-0 compute) so they don't steal HBM
            # bandwidth from the startup-critical wqk/x loads.
            wp_tiles = {}

            def mk_wp_load(ec):
                def emit():
                    wpt = cpool.tile([128, NCT, 512], BF16, name=f"wp{ec}")
                    nc.gpsimd.dma_start(
                        out=wpt[:],
                        in_=wproj[:, ec * 512 : (ec + 1) * 512].rearrange(
                            "(ft p) e -> p ft e", p=128
                        ),
                    )
                    wp_tiles[ec] = wpt
                return emit

            def mk_bproj_load():
                def emit():
                    nc.gpsimd.dma_start(out=bproj_sb[:], in_=bproj[:])
                return emit

            # a2a buffers: per batch, slot p carries my 2 heads' yT for
            # token strip p of that batch (128 tokens).
            a2a_in = [
                dpool.tile([NCORES, FV, STRIP], BF16, name=f"a2a_in{b}")
                for b in range(B)
            ]
            a2a_out = [
                dpool.tile([NCORES, FV, STRIP], BF16, name=f"a2a_out{b}")
                for b in range(B)
            ]

            qk_tiles = {}
            v_tiles = {}
            yts_tiles = {}

            # ---- QKV projection + RoPE: 16 emission units per batch ---
            def qkv_units(b):
                qkT_b = act.tile(
                    [128, 4, T], BF16, tag="qkT", name=f"qkT{b}"
                )
                v_b = act.tile(
                    [128, T // 128, FV], BF16, tag="vsb", name=f"v{b}"
                )
                qk_tiles[b] = qkT_b
                v_tiles[b] = v_b
                units = []
                for cc in range(2):
                    ch = 2 * b + cc
                    t0 = cc * 512

                    def mk_load(ch=ch):
                        def emit():
                            if ch in xt_tiles:
                                return xt_tiles[ch]
                            xT_ch = wpool.tile(
                                [128, NCT, 512], BF16, tag="xT_ch",
                                name=f"xT_ch{ch}",
                            )
                            xt_tiles[ch] = xT_ch
                            g0 = ch * 512
                            xT_r = xT[:, g0 : g0 + 512].rearrange(
                                "(ct p) t -> p ct t", p=128
                            )
                            for pc in range(2):
                                s = slice(pc * 8, (pc + 1) * 8)
                                nc.gpsimd.dma_start(
                                    out=xT_ch[:, s, :], in_=xT_r[:, s, :]
                                )
                            return xT_ch
                        return emit

                    load = mk_load()

                    def mk_mi(mi, load=load, t0=t0, qkT_b=qkT_b):
                        def emit():
                            xT_ch = load()
                            ps = psA.tile([128, 2, 512], F32, tag="mm512")
                            for ct in range(NCT):
                                nc.tensor.matmul(
                                    ps[:, 0, :],
                                    lhsT=wqk_sb[
                                        :, ct, mi * 128 : (mi + 1) * 128
                                    ],
                                    rhs=xT_ch[:, ct, :],
                                    start=(ct == 0),
                                    stop=(ct == NCT - 1),
                                )
                            # bias + RoPE: dst = (ps+b)*cos + R^T@((ps+b)*sin)
                            m1 = wpool.tile([128, 512], BF16, tag="rope_m1")
                            m2 = wpool.tile([128, 512], BF16, tag="rope_m2")
                            nc.vector.scalar_tensor_tensor(
                                out=m2[:], in0=ps[:, 0, :],
                                scalar=bqk_sb[:, mi : mi + 1],
                                in1=sin_sb[:, t0 : t0 + 512],
                                op0=add, op1=mult,
                            )
                            rot_ps = psB.tile([128, 512], F32, tag="aux")
                            nc.tensor.matmul(
                                rot_ps[:], lhsT=rmat_sb[:], rhs=m2[:],
                                start=True, stop=True,
                            )
                            nc.vector.scalar_tensor_tensor(
                                out=m1[:], in0=ps[:, 0, :],
                                scalar=bqk_sb[:, mi : mi + 1],
                                in1=cos_sb[:, t0 : t0 + 512],
                                op0=add, op1=mult,
                            )
                            nc.vector.tensor_add(
                                qkT_b[:, mi, t0 : t0 + 512], m1[:], rot_ps[:]
                            )
                        return emit

                    def mk_tt(tt, load=load, cc=cc, v_b=v_b):
                        def emit():
                            xT_ch = load()
                            psv = psB.tile([128, FV], F32, tag="acc")
                            for ct in range(NCT):
                                nc.tensor.matmul(
                                    psv[:],
                                    lhsT=xT_ch[
                                        :, ct, tt * 128 : (tt + 1) * 128
                                    ],
                                    rhs=wv_sb[:, ct, :],
                                    start=(ct == 0),
                                    stop=(ct == NCT - 1),
                                )
                            nc.vector.tensor_add(
                                v_b[:, cc * 4 + tt, :], psv[:], bv_sb[:]
                            )
                        return emit

                    for mi in range(4):
                        units.append(mk_mi(mi))
                    for tt in range(4):
                        units.append(mk_tt(tt))
                return units

            # ---- attention: 4 units per batch (hl, tqc); the last unit
            # issues the collective and the yts gather for proj ---------
            def att_units(b):
                def mk_unit(hl, tqc, last):
                    def emit():
                        qkT_b = qk_tiles[b]
                        v_b = v_tiles[b]
                        qh = qkT_b[:, hl, :]
                        kh = qkT_b[:, 2 + hl, :]
                        q0 = tqc * 512
                        nj = 4 * (tqc + 1)
                        ot_ps = psB.tile([128, 512], F32, tag="acc")
                        sacc = apool.tile([128, 512], F32, tag="sacc")
                        sacc_bf = apool.tile([128, 512], BF16, tag="saccb")
                        nadds = 0
                        for jp in range(nj // 2):
                            js = [2 * jp, 2 * jp + 1]
                            md = min(
                                max(0, 128 * (j - (nj - 4))) for j in js
                            )
                            st_ps = psA.tile(
                                [128, 2, 512], F32, tag="mm512"
                            )
                            for jj, j in enumerate(js):
                                s0 = j * 128
                                nc.tensor.matmul(
                                    st_ps[:, jj, md:512],
                                    lhsT=kh[:, s0 : s0 + 128],
                                    rhs=qh[:, q0 + md : q0 + 512],
                                    start=True,
                                    stop=True,
                                )
                            ptp = apool.tile(
                                [128, 2, 512], BF16, tag="pt", bufs=3
                            )
                            nc.scalar.activation(
                                ptp[:, :, md:512], st_ps[:, :, md:512],
                                Exp, scale=SCALE,
                            )
                            for jj, j in enumerate(js):
                                m = j - (nj - 4)
                                if m >= 0:
                                    nc.vector.tensor_mul(
                                        ptp[:, jj, :], ptp[:, jj, :],
                                        mask_sb[:, m, :],
                                    )
                            # DVE accumulation of exp chunks; the final
                            # add writes bf16 for the ones-matmul.
                            for jj in range(2):
                                nadds += 1
                                if nadds == 1:
                                    continue
                                if nadds == 2:
                                    nc.vector.tensor_add(
                                        sacc[:], ptp[:, 0, :], ptp[:, 1, :]
                                    )
                                elif nadds == nj:
                                    nc.vector.tensor_add(
                                        sacc_bf[:], sacc[:], ptp[:, jj, :]
                                    )
                                else:
                                    nc.vector.tensor_add(
                                        sacc[:], sacc[:], ptp[:, jj, :]
                                    )
                            for jj, j in enumerate(js):
                                vt = v_b[:, j, hl * 128 : (hl + 1) * 128]
                                nc.tensor.matmul(
                                    ot_ps[:], lhsT=vt, rhs=ptp[:, jj, :],
                                    start=(j == 0), stop=(j == nj - 1),
                                )
                        # partition-broadcast denominator in one matmul
                        den_ps = psB.tile([128, 512], F32, tag="aux")
                        nc.tensor.matmul(
                            den_ps[:], lhsT=ones_sb[:], rhs=sacc_bf[:],
                            start=True, stop=True,
                        )
                        recipb = apool.tile([128, 512], F32, tag="recipb")
                        nc.vector.reciprocal_approx_fast(recipb[:], den_ps[:])
                        yt = apool.tile([128, 512], BF16, tag="yt")
                        nc.vector.tensor_mul(yt[:], ot_ps[:], recipb[:])
                        # strips of 128 tokens -> a2a slots (one DMA)
                        dst = a2a_in[b][
                            tqc * 4 : (tqc + 1) * 4,
                            hl * 128 : (hl + 1) * 128,
                            :,
                        ].rearrange("s d t -> d s t")
                        nc.scalar.dma_start(
                            out=dst,
                            in_=yt[:].rearrange("d (s t) -> d s t", s=4),
                        )
                        if last:
                            nc.gpsimd.collective_compute(
                                "AllToAll",
                                mybir.AluOpType.bypass,
                                replica_groups=[list(range(NCORES))],
                                ins=[a2a_in[b][:].opt()],
                                outs=[a2a_out[b][:].opt()],
                            )
                            # gather my strip immediately (gpsimd queue is
                            # idle; later-emitted collectives would raise
                            # this DMA's wait threshold)
                            yts = wpool.tile(
                                [128, NCT, STRIP], BF16, tag="yts",
                                name=f"yts{b}", bufs=4,
                            )
                            yts_tiles[b] = yts
                            nc.sync.dma_start(
                                out=yts[:],
                                in_=a2a_out[b].rearrange(
                                    "g (f2 p) t -> p (g f2) t", p=128
                                ),
                            )
                    return emit

                units = []
                for hl in range(HPC):
                    for tqc in range(2):
                        units.append(
                            mk_unit(hl, tqc, hl == HPC - 1 and tqc == 1)
                        )
                return units

            # ---- projection of my 128-token strip: 4 units per batch --
            def proj_units(b):
                def mk_ec(ec):
                    def emit():
                        yts = yts_tiles[b]
                        e0 = ec * 512
                        pps = psA.tile([128, 2, 512], F32, tag="mm512")
                        for ft in range(NCT):
                            nc.tensor.matmul(
                                pps[:, 0, :],
                                lhsT=yts[:, ft, :],
                                rhs=wp_tiles[ec][:, ft, :],
                                start=(ft == 0),
                                stop=(ft == NCT - 1),
                            )
                        osb = wpool.tile([128, 512], F32, tag="osb")
                        nc.vector.tensor_add(
                            osb[:], pps[:, 0, :], bproj_sb[:, e0 : e0 + 512]
                        )
                        nc.scalar.dma_start(
                            out=out[
                                b * STRIP : (b + 1) * STRIP, e0 : e0 + 512
                            ],
                            in_=osb[:],
                        )
                    return emit

                return [mk_ec(ec) for ec in range(4)]

            # ---- schedule: interleave fillers 1-per-2 qkv groups ------
            def interleave(groups, fillers):
                fi = 0
                for gi, g in enumerate(groups):
                    g()
                    if gi % 2 == 1 and fi < len(fillers):
                        fillers[fi]()
                        fi += 1
                for f in fillers[fi:]:
                    f()

            # all proj work is deferred to the tail so ~39us of PE work
            # is queued behind the last collective's doorbell, hiding its
            # ~38us latency.
            wp_loads = [mk_wp_load(ec) for ec in range(4)] + [mk_bproj_load()]
            interleave(qkv_units(0), [])
            interleave(qkv_units(1), att_units(0) + wp_loads)
            interleave(qkv_units(2), att_units(1))
            interleave(qkv_units(3), att_units(2))
            a3 = att_units(3)
            p0 = proj_units(0)
            tail = [
                a3[0], p0[0], a3[1], p0[1], a3[2], p0[2], a3[3], p0[3],
            ] + proj_units(1) + proj_units(2) + proj_units(3)
            interleave([], tail)

    nc.compile()
    return nc


def _rope_tables():
    inv = 1.0 / (10000.0 ** (np.arange(0, D, 2, dtype=np.float64) / D))
    t = np.arange(T, dtype=np.float64)
    fr = np.outer(t, inv)  # [T, 64]
    cosT = np.tile(np.cos(fr).T, (2, 1)).astype(ml_dtypes.bfloat16)
    sinT = np.tile(np.sin(fr).T, (2, 1)).astype(ml_dtypes.bfloat16)
    return np.ascontiguousarray(cosT), np.ascontiguousarray(sinT)


def _prep_inputs(x, Wqkv, bqkv, Wproj, bproj):
    x = np.asarray(x, np.float32).reshape(TQ, C)
    Wqkv = np.asarray(Wqkv, np.float32)
    bqkv = np.asarray(bqkv, np.float32)
    Wproj = np.ascontiguousarray(
        np.asarray(Wproj, np.float32).astype(ml_dtypes.bfloat16)
    )
    bproj = np.asarray(bproj, np.float32)

    xT = np.ascontiguousarray(x.T.astype(ml_dtypes.bfloat16))
    cosT, sinT = _rope_tables()
    rmat = np.zeros((128, 128), ml_dtypes.bfloat16)
    for i in range(64):
        rmat[64 + i, i] = -1.0   # out[p<64]  = -m2[p+64]
        rmat[i, 64 + i] = 1.0    # out[p>=64] = +m2[p-64]
    bproj_b = np.ascontiguousarray(np.broadcast_to(bproj[None, :], (128, C)))

    Wq = Wqkv[:, 0 * C : 1 * C].reshape(C, H, D)
    Wk = Wqkv[:, 1 * C : 2 * C].reshape(C, H, D)
    Wv = Wqkv[:, 2 * C : 3 * C].reshape(C, H, D)
    bq = bqkv[0 * C : 1 * C].reshape(H, D)
    bk = bqkv[1 * C : 2 * C].reshape(H, D)
    bv = bqkv[2 * C : 3 * C].reshape(H, D)

    in_maps = []
    for r in range(NCORES):
        ha, hb = 2 * r, 2 * r + 1
        wqk_s = np.ascontiguousarray(
            np.concatenate(
                [Wq[:, ha], Wq[:, hb], Wk[:, ha], Wk[:, hb]], axis=1
            ).astype(ml_dtypes.bfloat16)
        )
        bqk_s = np.ascontiguousarray(
            np.stack([bq[ha], bq[hb], bk[ha], bk[hb]], axis=1)
        )  # [128, 4]
        wv_s = np.ascontiguousarray(
            np.concatenate([Wv[:, ha], Wv[:, hb]], axis=1).astype(
                ml_dtypes.bfloat16
            )
        )
        bv_s = np.ascontiguousarray(
            np.broadcast_to(
                np.concatenate([bv[ha], bv[hb]])[None, :], (128, FV)
            )
        )
        in_maps.append(
            {
                "xT": xT,
                "wqk": wqk_s,
                "wv": wv_s,
                "bqk": bqk_s,
                "bv": bv_s,
                "wproj": Wproj,
                "bproj": bproj_b,
                "cosd": cosT,
                "sind": sinT,
                "rmat": rmat,
            }
        )
    return in_maps


def kernel(x, Wqkv, bqkv, Wproj, bproj, _trace=False, _trace_kwargs=None):
    if "nc" not in _CACHE:
        _CACHE["nc"] = _build_program()
    nc = _CACHE["nc"]
    in_maps = _prep_inputs(x, Wqkv, bqkv, Wproj, bproj)
    kwargs = {}
    if _trace:
        kwargs.update(trace=True, **(_trace_kwargs or {}))
    res = run_bass_kernel_spmd(nc, in_maps, core_ids=list(range(NCORES)), **kwargs)
    _CACHE["last_results"] = res
    # core r's out: [4*128, C]; row block b = tokens [b*T + r*128, +128)
    full = np.stack(
        [res.results[r]["out"].reshape(B, STRIP, C) for r in range(NCORES)],
        axis=1,
    )  # [B, NCORES, STRIP, C]
    return np.ascontiguousarray(
        full.reshape(B, T, C).astype(np.float32)
    )
